# revision 14
# baseline (speedup 1.0000x reference)
import sys

sys.path.insert(0, "/opt/trn_rl_repo")
import numpy as np
import ml_dtypes

from concourse import bass, mybir
from concourse.tile import TileContext
from concourse.masks import make_identity

# Workaround: walrus in this container rejects the TileContext exit Drain when
# it carries many sem waits ("Too many sync wait commands"). Split the waits
# onto one NOP per proc, then drain with no waits.
import bass_rust as _br
from concourse.tile_sem_assignment import N_PROCS
import concourse.tile as _tile


def _patched_drain_and_barrier(self, tick_clock, wait_clock):
    nc = self.nc
    g = tick_clock.global_clock
    for p in range(N_PROCS):
        if g[p] == 0:
            continue
        partial = _br.VectorClock([g[q] if q == p else 0 for q in range(N_PROCS)])
        nop = nc.sync.nop(nofuse=True, hint=f"drain_wait_p{p}")
        wait_clock.add_sem_waits(nop.ins, _br.ScopedClock({None: partial}))
    nc.sync.drain()
    nc.all_engine_barrier()
    assert self.sems is not None
    popped = nc._tile_sem_poison_stack.pop()
    assert popped is self._sem_poison
    nc.clear_and_free_semaphores(list(self.sems.allocated().values()))
    nc.all_engine_barrier()


_tile.TileContext._drain_and_barrier = _patched_drain_and_barrier

B, N, K, D, H, DH, T, HID = 32, 4096, 8, 256, 4, 64, 3, 256
SCALE = DH ** -0.5
EPS = 1e-8
LNEPS = 1e-5
NCORES = 8
BL = B // NCORES  # batches per core
NB = N // 128  # 32 n-blocks of 128

F32 = mybir.dt.float32
BF16 = mybir.dt.bfloat16
AF = mybir.ActivationFunctionType
ALU = mybir.AluOpType
AX = mybir.AxisListType


def _view(ap, free_dims):
    """Re-view an AP's free dims (list of [stride, num]) keeping its partition dim."""
    return bass.AP(tensor=ap.tensor, offset=ap.offset, ap=[ap.ap[0]] + free_dims)


def build_program():
    nc = bass.Bass()
    x_ext = nc.declare_dram_parameter("x", [BL * N, D], F32, isOutput=False)
    s0_ext = nc.declare_dram_parameter("slots0", [BL * K, D], F32, isOutput=False)
    wq_ext = nc.declare_dram_parameter("wq", [128, 2, D], BF16, isOutput=False)
    wk_ext = nc.declare_dram_parameter("wk", [128, 2, D], BF16, isOutput=False)
    wv_ext = nc.declare_dram_parameter("wv", [128, 2, D], BF16, isOutput=False)
    wo_ext = nc.declare_dram_parameter("wo", [128, 2, D], BF16, isOutput=False)
    wih_ext = nc.declare_dram_parameter("wih", [128, 2, 3 * D], BF16, isOutput=False)
    whh_ext = nc.declare_dram_parameter("whh", [128, 2, 3 * D], BF16, isOutput=False)
    w1_ext = nc.declare_dram_parameter("w1", [128, 2, HID], BF16, isOutput=False)
    w2_ext = nc.declare_dram_parameter("w2", [128, 2, D], BF16, isOutput=False)
    out_ext = nc.declare_dram_parameter("out", [BL * K, D], F32, isOutput=True)

    with TileContext(nc) as tc:
        from contextlib import ExitStack

        with ExitStack() as ctx:
            persist = ctx.enter_context(tc.tile_pool(name="persist", bufs=1))

            wq = persist.tile([128, 2, D], BF16)
            wk = persist.tile([128, 2, D], BF16)
            wv = persist.tile([128, 2, D], BF16)
            wo = persist.tile([128, 2, D], BF16)
            wih = persist.tile([128, 2, 3 * D], BF16)
            whh = persist.tile([128, 2, 3 * D], BF16)
            w1 = persist.tile([128, 2, HID], BF16)
            w2 = persist.tile([128, 2, D], BF16)
            for t, e in [(wq, wq_ext), (wk, wk_ext), (wv, wv_ext), (wo, wo_ext),
                         (wih, wih_ext), (whh, whh_ext), (w1, w1_ext), (w2, w2_ext)]:
                nc.sync.dma_start(out=t[:], in_=e[:])

            ident = persist.tile([128, 128], BF16)
            make_identity(nc, ident[:])
            ones_col = persist.tile([128, 1], BF16)
            nc.vector.memset(ones_col[:], 1.0)
            ones_row = persist.tile([1, 128], F32)
            nc.vector.memset(ones_row[:], 1.0)
            zero128 = persist.tile([128, 1], F32)
            nc.vector.memset(zero128[:], 0.0)
            eps128 = persist.tile([128, 1], F32)
            nc.vector.memset(eps128[:], LNEPS)
            nc.const_aps.aps[(F32, 0.0)] = zero128[:]
            nc.const_aps.aps[(F32, LNEPS)] = eps128[:]

            kfT = [persist.tile([128, 2, N], BF16, name=f"kfT{b}")
                   for b in range(BL)]
            vf = [persist.tile([128, NB, D], BF16, name=f"vf{b}")
                  for b in range(BL)]

            slots_init = persist.tile([BL * K, D], F32)
            nc.sync.dma_start(out=slots_init[:], in_=s0_ext[:])

            # ---------------- Phase A: LN(x), k/v features ----------------
            pA = ctx.enter_context(tc.tile_pool(name="pA", bufs=3))
            with tc.tile_pool(name="psA", bufs=2, space="PSUM") as psA:
                for b in range(BL):
                    for nb in range(NB):
                        r0 = b * N + nb * 128
                        xt = pA.tile([128, D], F32)
                        nc.sync.dma_start(out=xt[:], in_=x_ext[r0:r0 + 128, :])
                        st = pA.tile([128, 6], F32)
                        nc.vector.bn_stats(out=st[:], in_=xt[:])
                        mv = pA.tile([128, 2], F32)
                        nc.vector.bn_aggr(out=mv[:], in_=st[:])
                        sq = pA.tile([128, 1], F32)
                        nc.scalar.activation(out=sq[:], in_=mv[:, 1:2], func=AF.Sqrt,
                                             bias=LNEPS)
                        rstd = pA.tile([128, 1], F32)
                        nc.vector.reciprocal(out=rstd[:], in_=sq[:])
                        xn = pA.tile([128, D], BF16)
                        nc.vector.tensor_scalar(out=xn[:], in0=xt[:],
                                                scalar1=mv[:, 0:1], scalar2=rstd[:],
                                                op0=ALU.subtract, op1=ALU.mult)
                        xnT_ps = psA.tile([128, 2, 128], BF16)
                        for c in range(2):
                            nc.tensor.transpose(out=xnT_ps[:, c, :],
                                                in_=xn[:, c * 128:(c + 1) * 128],
                                                identity=ident[:])
                        xnT = pA.tile([128, 2, 128], BF16)
                        nc.vector.tensor_copy(out=xnT[:], in_=xnT_ps[:])
                        # kT = Wk_eff @ xn.T  (accumulate over feature chunks c)
                        kps = psA.tile([128, 2, 128], F32)
                        for dc in range(2):
                            for c in range(2):
                                nc.tensor.matmul(out=kps[:, dc, :],
                                                 lhsT=wk[:, c, dc * 128:(dc + 1) * 128],
                                                 rhs=xnT[:, c, :],
                                                 start=(c == 0), stop=(c == 1))
                        nc.scalar.copy(out=kfT[b][:, :, nb * 128:(nb + 1) * 128],
                                       in_=kps[:])
                        # v = xn @ Wv_eff.T  ([n, d] layout)
                        vps = psA.tile([128, D], F32)
                        for c in range(2):
                            nc.tensor.matmul(out=vps[:], lhsT=xnT[:, c, :],
                                             rhs=wv[:, c, :],
                                             start=(c == 0), stop=(c == 1))
                        nc.gpsimd.tensor_copy(out=vf[b][:, nb, :], in_=vps[:])

            # ---------------- Phase B: 3 slot-attention steps ----------------
            pB = ctx.enter_context(tc.tile_pool(name="pB", bufs=2))
            pAt = ctx.enter_context(tc.tile_pool(name="pAt", bufs=4))
            slp = ctx.enter_context(tc.tile_pool(name="slp", bufs=2))

            def tr32(src, psQ):
                # src [32, 256] bf16 -> [128, 2, 32] bf16 (transpose via PE)
                tp = psQ.tile([128, 2, 32], BF16, name="sm")
                for c in range(2):
                    nc.tensor.transpose(out=tp[:, c, :],
                                        in_=src[:, c * 128:(c + 1) * 128],
                                        identity=ident[0:32, 0:32])
                r = pB.tile([128, 2, 32], BF16)
                nc.scalar.copy(out=r[:], in_=tp[:])
                return r

            def ln32(src):
                # layernorm (no affine) of [32, 256] f32 -> bf16
                st = pB.tile([32, 6], F32)
                nc.vector.bn_stats(out=st[:], in_=src[:])
                mv = pB.tile([32, 2], F32)
                nc.vector.bn_aggr(out=mv[:], in_=st[:])
                sq = pB.tile([32, 1], F32)
                nc.scalar.activation(out=sq[:], in_=mv[:, 1:2], func=AF.Sqrt,
                                     bias=LNEPS)
                rstd = pB.tile([32, 1], F32)
                nc.vector.reciprocal(out=rstd[:], in_=sq[:])
                ln = pB.tile([32, D], BF16)
                nc.vector.tensor_scalar(out=ln[:], in0=src[:], scalar1=mv[:, 0:1],
                                        scalar2=rstd[:], op0=ALU.subtract,
                                        op1=ALU.mult)
                return ln

            with tc.tile_pool(name="psD", bufs=2, space="PSUM") as psD, \
                 tc.tile_pool(name="psQ", bufs=4, space="PSUM") as psQ:
                cur = slots_init
                for step in range(T):
                    ln_s = ln32(cur)
                    lnT = tr32(ln_s, psQ)
                    # qT [128, 2, 32]: rows d, cols (b*8+k)
                    qps = psQ.tile([128, 2, 32], F32, name="sm")
                    for dc in range(2):
                        for c in range(2):
                            nc.tensor.matmul(out=qps[:, dc, :],
                                             lhsT=wq[:, c, dc * 128:(dc + 1) * 128],
                                             rhs=lnT[:, c, :],
                                             start=(c == 0), stop=(c == 1))
                    # q_bd [128, 2, 64] bf16: per dc, cols b*16 + hl*8 + k
                    qbd = pB.tile([128, 2, 4 * 16], BF16)
                    nc.vector.memset(qbd[:], 0.0)
                    for dc in range(2):
                        for b in range(BL):
                            nc.gpsimd.tensor_copy(
                                out=qbd[0:64, dc, b * 16:b * 16 + 8],
                                in_=qps[0:64, dc, b * 8:b * 8 + 8])
                            nc.gpsimd.tensor_copy(
                                out=qbd[64:128, dc, b * 16 + 8:b * 16 + 16],
                                in_=qps[64:128, dc, b * 8:b * 8 + 8])

                    # dots + softmax per batch
                    attns = []
                    for b in range(BL):
                        dps = psD.tile([128, NB, 32], F32, name="big")
                        for nb in range(NB):
                            for dc in range(2):
                                nc.tensor.matmul(
                                    out=dps[:, nb, dc * 16:(dc + 1) * 16],
                                    lhsT=kfT[b][:, dc, nb * 128:(nb + 1) * 128],
                                    rhs=qbd[:, dc, b * 16:(b + 1) * 16],
                                    start=True, stop=True)
                        ex = pB.tile([128, 4 * NB, 8], F32)
                        nc.scalar.activation(
                            out=ex[:],
                            in_=_view(dps[:], [[8, 4 * NB], [1, 8]]),
                            func=AF.Exp, scale=SCALE)
                        Z = pB.tile([128, 4 * NB], F32)
                        nc.vector.tensor_reduce(out=Z[:], in_=ex[:], axis=AX.X,
                                                op=ALU.add)
                        Zr = pB.tile([128, 4 * NB], F32)
                        nc.vector.reciprocal(out=Zr[:], in_=Z[:])
                        attn = pAt.tile([128, 4 * NB, 8], BF16)
                        nc.vector.tensor_tensor(
                            out=attn[:], in0=ex[:],
                            in1=_view(Zr[:], [[1, 4 * NB], [0, 8]]),
                            op=ALU.mult)
                        attns.append(attn)

                    # weighted sums + renorm, all batches
                    updT = pB.tile([128, 2, BL * K], BF16)
                    for b in range(BL):
                        upsS = psQ.tile([128, 3, 32], F32, name="sm")
                        for dc in range(2):
                            for nb in range(NB):
                                nc.tensor.matmul(
                                    out=upsS[:, dc, :],
                                    lhsT=vf[b][:, nb, dc * 128:(dc + 1) * 128],
                                    rhs=attns[b][:, nb * 4:(nb + 1) * 4, :],
                                    start=(nb == 0), stop=(nb == NB - 1))
                        for nb in range(NB):
                            nc.tensor.matmul(out=upsS[0:1, 2, :],
                                             lhsT=ones_col[:],
                                             rhs=attns[b][:, nb * 4:(nb + 1) * 4, :],
                                             start=(nb == 0), stop=(nb == NB - 1))
                        Se = pB.tile([1, 32], F32)
                        nc.vector.tensor_scalar_add(out=Se[:], in0=upsS[0:1, 2, :],
                                                    scalar1=EPS)
                        Sr = pB.tile([1, 32], F32)
                        nc.vector.reciprocal(out=Sr[:], in_=Se[:])
                        sbps = psQ.tile([128, 32], F32, name="sm")
                        nc.tensor.matmul(out=sbps[:], lhsT=ones_row[:], rhs=Sr[:],
                                         start=True, stop=True)
                        Sbc = pB.tile([128, 32], F32)
                        nc.gpsimd.tensor_copy(out=Sbc[:], in_=sbps[:])
                        for dc in range(2):
                            hA, hB = dc * 16, dc * 16 + 8
                            nc.vector.tensor_tensor(
                                out=updT[0:64, dc, b * 8:b * 8 + 8],
                                in0=upsS[0:64, dc, hA:hA + 8],
                                in1=Sbc[0:64, hA:hA + 8], op=ALU.mult)
                            nc.vector.tensor_tensor(
                                out=updT[64:128, dc, b * 8:b * 8 + 8],
                                in0=upsS[64:128, dc, hB:hB + 8],
                                in1=Sbc[64:128, hB:hB + 8], op=ALU.mult)

                    # upd @ Wo.T  -> [32, 256]
                    wops = psQ.tile([32, D], F32, name="sm")
                    for dc in range(2):
                        nc.tensor.matmul(out=wops[:], lhsT=updT[:, dc, :],
                                         rhs=wo[:, dc, :],
                                         start=(dc == 0), stop=(dc == 1))
                    upd_sb = pB.tile([32, D], BF16)
                    nc.scalar.copy(out=upd_sb[:], in_=wops[:])
                    updfT = tr32(upd_sb, psQ)

                    # GRU
                    sc = pB.tile([BL * K, D], BF16)
                    nc.scalar.copy(out=sc[:], in_=cur[:])
                    sT = tr32(sc, psQ)
                    gips = psD.tile([32, 2, 512], F32, name="big")
                    ghps = psD.tile([32, 2, 512], F32, name="big")
                    for dc in range(2):
                        nc.tensor.matmul(out=gips[:, 0, :], lhsT=updfT[:, dc, :],
                                         rhs=wih[:, dc, 0:512],
                                         start=(dc == 0), stop=(dc == 1))
                        nc.tensor.matmul(out=gips[:, 1, 0:256], lhsT=updfT[:, dc, :],
                                         rhs=wih[:, dc, 512:768],
                                         start=(dc == 0), stop=(dc == 1))
                        nc.tensor.matmul(out=ghps[:, 0, :], lhsT=sT[:, dc, :],
                                         rhs=whh[:, dc, 0:512],
                                         start=(dc == 0), stop=(dc == 1))
                        nc.tensor.matmul(out=ghps[:, 1, 0:256], lhsT=sT[:, dc, :],
                                         rhs=whh[:, dc, 512:768],
                                         start=(dc == 0), stop=(dc == 1))
                    rz_in = pB.tile([32, 512], F32)
                    nc.vector.tensor_tensor(out=rz_in[:], in0=gips[:, 0, :],
                                            in1=ghps[:, 0, :], op=ALU.add)
                    rz = pB.tile([32, 512], F32)
                    nc.scalar.activation(out=rz[:], in_=rz_in[:], func=AF.Sigmoid)
                    nt = pB.tile([32, D], F32)
                    nc.vector.tensor_tensor(out=nt[:], in0=rz[:, 0:256],
                                            in1=ghps[:, 1, 0:256], op=ALU.mult)
                    nt2 = pB.tile([32, D], F32)
                    nc.vector.tensor_tensor(out=nt2[:], in0=nt[:],
                                            in1=gips[:, 1, 0:256], op=ALU.add)
                    nn = pB.tile([32, D], F32)
                    nc.scalar.activation(out=nn[:], in_=nt2[:], func=AF.Tanh)
                    d1 = pB.tile([32, D], F32)
                    nc.vector.tensor_tensor(out=d1[:], in0=cur[:], in1=nn[:],
                                            op=ALU.subtract)
                    d2 = pB.tile([32, D], F32)
                    nc.vector.tensor_tensor(out=d2[:], in0=rz[:, 256:512], in1=d1[:],
                                            op=ALU.mult)
                    s_g = slp.tile([32, D], F32)
                    nc.vector.tensor_tensor(out=s_g[:], in0=nn[:], in1=d2[:],
                                            op=ALU.add)

                    # MLP with residual
                    ln2 = ln32(s_g)
                    ln2T = tr32(ln2, psQ)
                    h1ps = psQ.tile([32, HID], F32, name="sm")
                    for dc in range(2):
                        nc.tensor.matmul(out=h1ps[:], lhsT=ln2T[:, dc, :],
                                         rhs=w1[:, dc, :],
                                         start=(dc == 0), stop=(dc == 1))
                    h1 = pB.tile([32, HID], BF16)
                    nc.scalar.activation(out=h1[:], in_=h1ps[:], func=AF.Relu)
                    h1T = tr32(h1, psQ)
                    mps = psQ.tile([32, D], F32, name="sm")
                    for dc in range(2):
                        nc.tensor.matmul(out=mps[:], lhsT=h1T[:, dc, :],
                                         rhs=w2[:, dc, :],
                                         start=(dc == 0), stop=(dc == 1))
                    s_new = slp.tile([32, D], F32)
                    nc.vector.tensor_tensor(out=s_new[:], in0=s_g[:], in1=mps[:],
                                            op=ALU.add)
                    cur = s_new

                nc.sync.dma_start(out=out_ext[:], in_=cur[:])
    return nc


def _wT(w):
    """[out_dim, in_dim] weight -> lhsT layout [128, in_dim//128, out_dim] bf16."""
    o, i = w.shape
    r = np.ascontiguousarray(w.T.reshape(i // 128, 128, o).transpose(1, 0, 2))
    return r.astype(ml_dtypes.bfloat16)


def prepare_inputs(inputs):
    """Full numpy inputs -> per-core in_maps list."""
    f = {k: np.asarray(v, dtype=np.float32) for k, v in inputs.items()}
    for nm in ("norm_in_b", "norm_slot_b", "norm_mlp_b", "gru_bih", "gru_bhh",
               "mlp_b1", "mlp_b2"):
        if np.max(np.abs(f[nm])) != 0.0:
            raise NotImplementedError(f"nonzero {nm} not supported")

    slots0 = f["mu"].reshape(1, 1, D) + np.exp(f["log_sigma"].reshape(1, 1, D)) \
        * f["noise"].reshape(B, K, D)  # [B, K, D]

    wq = _wT(f["Wq"] * f["norm_slot_w"][None, :])
    wk = _wT(f["Wk"] * f["norm_in_w"][None, :])
    wv = _wT(f["Wv"] * f["norm_in_w"][None, :])
    wo = _wT(f["Wo"])
    wih = _wT(f["gru_Wih"])
    whh = _wT(f["gru_Whh"])
    w1 = _wT(f["mlp_W1"] * f["norm_mlp_w"][None, :])
    w2 = _wT(f["mlp_W2"])

    x = f["x"]  # [B, N, D]
    in_maps = []
    for c in range(NCORES):
        b0 = c * BL
        in_maps.append({
            "x": np.ascontiguousarray(x[b0:b0 + BL].reshape(BL * N, D)),
            "slots0": np.ascontiguousarray(
                slots0[b0:b0 + BL].reshape(BL * K, D).astype(np.float32)),
            "wq": wq, "wk": wk, "wv": wv, "wo": wo,
            "wih": wih, "whh": whh, "w1": w1, "w2": w2,
        })
    return in_maps


_CACHED_NC = None


def kernel(**inputs):
    global _CACHED_NC
    from concourse.bass_utils import run_bass_kernel_spmd
    in_maps = prepare_inputs(inputs)
    if _CACHED_NC is None:
        _CACHED_NC = build_program()
    res = run_bass_kernel_spmd(_CACHED_NC, in_maps, core_ids=list(range(NCORES)),
                               trace=False)
    outs = [np.asarray(res.results[c]["out"], dtype=np.float32).reshape(BL, K, D)
            for c in range(NCORES)]
    return np.concatenate(outs, axis=0)
